# revision 1
# baseline (speedup 1.0000x reference)
"""Trainium2 Bass kernel for nn_HarmonicOscillatorOrbitals.

out[b, i, j] = exp(-s^2/2) * H_j(s), s = omega * x[b, i, 0], j = 0..31
(physicists' Hermite polynomials), data-parallel over 8 NeuronCores on
the leading batch axis.

Per core (8192 batches = 262144 scalars as [128 partitions, E=2048]):
  t   = 2*omega*x
  env = exp(-s^2/2) = 2^(t^2 * -log2(e)/8), computed exactly on DVE:
        2^n by float-magic + integer exponent shift, 2^f by a degree-5
        polynomial (fused scalar_tensor_tensor Horner chain) — the ACT
        spline Exp is ~1e-5 off, this path is ~1e-6.
  G_0 = env, G_1 = t*env, G_k = t*G_{k-1} - 2(k-1)*G_{k-2}  (= env*H_k)

The per-element recurrence is serial in k, so elements are split into
two independent column slices: DVE runs one chain (tensor_mul + fused
scalar_tensor_tensor), GPSIMD the other (tensor_mul + tensor_sub, with
ACT supplying the 2(k-1)*G_{k-2} scale-copies two steps ahead). Each
chain keeps its engine fully busy with no cross-engine ping-pong.

G_k slices stay contiguous in SBUF; DRAM output is k-major
[128, 32, E] (1.3KB DMA descriptors) and the host permutes to
(batch, i, j) while unsharding.
"""

from contextlib import ExitStack

import numpy as np

import concourse.bacc as bacc
import concourse.mybir as mybir
import concourse.tile as tile
from concourse.bass_utils import run_bass_kernel_spmd

F32 = mybir.dt.float32
I32 = mybir.dt.int32
AF = mybir.ActivationFunctionType
ALU = mybir.AluOpType

NJ = 32          # number of Hermite orders
N_CORES = 8
B = 65536        # full batch
BC = B // N_CORES
E = BC * NJ // 128   # 2048 free elems per partition per core

TILE_F = 512     # columns per tile
FD = 300         # DVE-owned columns per tile (rest on GPSIMD)

# exp2: env = 2^v, v = sq * K4 with sq = t^2 = 4 s^2
K4 = float(np.float32(-np.log2(np.e) / 8.0))
MAGIC = float(np.float32(1.5 * 2**23))
EXP_B4, EXP_B3, EXP_B2, EXP_B1 = 7.292242, 41.85769, 181.15059, 522.6992
EXP_A5, EXP_A0 = 0.0013260915, 1.0
EXP_EIMM = 127 - 0x4B400000  # (bits(w) + EXP_EIMM) << 23 == bits(2^n)


def _build(e=E, tile_f=TILE_F, fd=FD, accurate_env=False):
    nc = bacc.Bacc("TRN2", target_bir_lowering=False, debug=False)
    x_d = nc.dram_tensor("x", [128, e], F32, kind="ExternalInput").ap()
    om_d = nc.dram_tensor("om", [1, 1], F32, kind="ExternalInput").ap()
    # raw tile dump: per f-tile, the DVE-slice tile [128, NJ*fd] then the
    # GPSIMD-slice tile [128, NJ*fg], verbatim — host unscrambles
    out_d = nc.dram_tensor("out", [128, NJ * e], F32, kind="ExternalOutput").ap()

    fg = tile_f - fd
    n_tiles = e // tile_f
    with tile.TileContext(nc) as tc, ExitStack() as ctx:
        cpool = ctx.enter_context(tc.tile_pool(name="const", bufs=1))
        xp = ctx.enter_context(tc.tile_pool(name="xp", bufs=4))
        ep = ctx.enter_context(tc.tile_pool(name="ep", bufs=3))
        qd = ctx.enter_context(tc.tile_pool(name="qd", bufs=4))
        qg = ctx.enter_context(tc.tile_pool(name="qg", bufs=4))
        gdp = ctx.enter_context(tc.tile_pool(name="gdp", bufs=2))
        ggp = ctx.enter_context(tc.tile_pool(name="ggp", bufs=2))

        om1 = cpool.tile([128, 1], F32)
        nc.sync.dma_start(om1[0:1, :], om_d[:, :])
        om2 = cpool.tile([128, 1], F32)
        nc.gpsimd.partition_broadcast(om2[:, :], om1[0:1, :])
        nc.scalar.mul(om2[:, :], om2[:, :], 2.0)  # om2 = 2*omega

        # int32 constants for the exponent fixup: (bits(w) + EXP_EIMM) << 23
        addc = cpool.tile([128, tile_f], I32)
        nc.vector.memset(addc[:, :], EXP_EIMM)
        t23 = cpool.tile([128, tile_f], I32)
        nc.vector.memset(t23[:, :], 23)

        for it in range(n_tiles):
            f0 = it * tile_f
            x_t = xp.tile([128, tile_f], F32)
            nc.sync.dma_start(x_t[:, :], x_d[:, f0 : f0 + tile_f])
            t_t = xp.tile([128, tile_f], F32, tag="t")
            nc.scalar.mul(t_t[:, :], x_t[:, :], om2[:, 0:1])  # t = 2*omega*x

            # four k-quarters per slice: DMA each out as soon as its 8
            # columns are done, so pool slots recycle at 1/4-tile grain
            gd_q = [
                gdp.tile([128, 8 * fd], F32, name=f"gdq{q}_{it}", tag=f"gd{q}")
                for q in range(4)
            ]
            gg_q = [
                ggp.tile([128, 8 * fg], F32, name=f"ggq{q}_{it}", tag=f"gg{q}")
                for q in range(4)
            ]

            def gds(k):
                return gd_q[k // 8][:, (k % 8) * fd : (k % 8 + 1) * fd]

            def ggs(k):
                return gg_q[k // 8][:, (k % 8) * fg : (k % 8 + 1) * fg]

            base = it * NJ * tile_f

            def flush_quarter(q):
                nc.sync.dma_start(
                    out_d[:, base + q * 8 * fd : base + (q + 1) * 8 * fd],
                    gd_q[q][:, :],
                )
                goff = base + NJ * fd
                nc.sync.dma_start(
                    out_d[:, goff + q * 8 * fg : goff + (q + 1) * 8 * fg],
                    gg_q[q][:, :],
                )

            if accurate_env:
                # ---- exact exp2 on DVE, full tile width ----
                sq = ep.tile([128, tile_f], F32, tag="sq")
                nc.vector.tensor_mul(sq[:, :], t_t[:, :], t_t[:, :])
                v_t = ep.tile([128, tile_f], F32, tag="v")
                nc.vector.tensor_scalar_mul(v_t[:, :], sq[:, :], K4)
                w_t = ep.tile([128, tile_f], F32, tag="w")
                nc.vector.tensor_scalar_add(w_t[:, :], v_t[:, :], MAGIC)
                n_t = ep.tile([128, tile_f], F32, tag="n")
                nc.vector.tensor_scalar_sub(n_t[:, :], w_t[:, :], MAGIC)
                f_t = ep.tile([128, tile_f], F32, tag="f")
                nc.vector.tensor_sub(f_t[:, :], v_t[:, :], n_t[:, :])
                p_t = ep.tile([128, tile_f], F32, tag="p")
                nc.vector.scalar_tensor_tensor(
                    p_t[:, :], f_t[:, :], EXP_B4, f_t[:, :], ALU.add, ALU.mult
                )
                for bb in (EXP_B3, EXP_B2, EXP_B1):
                    nc.vector.scalar_tensor_tensor(
                        p_t[:, :], p_t[:, :], bb, f_t[:, :], ALU.add, ALU.mult
                    )
                nc.vector.tensor_scalar(
                    p_t[:, :], p_t[:, :], EXP_A5, EXP_A0, ALU.mult, ALU.add
                )
                e2_t = ep.tile([128, tile_f], I32, tag="e2")
                nc.vector.tensor_tensor(
                    e2_t[:, :], w_t[:, :].bitcast(I32), addc[:, :], ALU.add
                )
                nc.vector.tensor_tensor(
                    e2_t[:, :], e2_t[:, :], t23[:, :], ALU.logical_shift_left
                )
                e2f = e2_t[:, :].bitcast(F32)
                # env split straight into the two G tiles (k = 0)
                nc.vector.tensor_mul(gds(0), p_t[:, 0:fd], e2f[:, 0:fd])
                nc.vector.tensor_mul(ggs(0), p_t[:, fd:], e2f[:, fd:])
            else:
                sq = ep.tile([128, tile_f], F32, tag="sq")
                nc.scalar.activation(sq[:, :], t_t[:, :], AF.Square, scale=0.5)
                nc.scalar.activation(gds(0), sq[:, 0:fd], AF.Exp, scale=-0.5)
                nc.scalar.activation(ggs(0), sq[:, fd:], AF.Exp, scale=-0.5)

            # G_1 = t * env, each engine seeds its own chain
            nc.vector.tensor_mul(gds(1), t_t[:, 0:fd], gds(0))
            nc.gpsimd.tensor_mul(ggs(1), t_t[:, fd:], ggs(0))

            for k in range(2, NJ):
                c = 2.0 * (k - 1)
                # DVE chain
                q_t = qd.tile([128, fd], F32)
                nc.vector.tensor_mul(q_t[:, :], t_t[:, 0:fd], gds(k - 1))
                nc.vector.scalar_tensor_tensor(
                    gds(k), gds(k - 2), -c, q_t[:, :], ALU.mult, ALU.add
                )
                # GPSIMD chain (ACT supplies c*G_{k-2})
                qg_t = qg.tile([128, fg], F32)
                rg_t = qg.tile([128, fg], F32, tag="rg")
                nc.gpsimd.tensor_mul(qg_t[:, :], t_t[:, fd:], ggs(k - 1))
                nc.scalar.mul(rg_t[:, :], ggs(k - 2), c)
                nc.gpsimd.tensor_sub(ggs(k), qg_t[:, :], rg_t[:, :])
                if k % 8 == 7:
                    flush_quarter(k // 8)

    nc.compile()
    return nc


_CACHED_NC = None


def kernel(x: np.ndarray, omega_kernel: np.ndarray, **run_kwargs) -> np.ndarray:
    global _CACHED_NC
    assert x.shape == (B, NJ, 1) and omega_kernel.shape == (1, 1), (
        x.shape,
        omega_kernel.shape,
    )
    x = np.ascontiguousarray(x, np.float32)
    om = np.ascontiguousarray(omega_kernel, np.float32)

    if _CACHED_NC is None:
        _CACHED_NC = _build()
    nc = _CACHED_NC

    in_maps = [
        {
            "x": x[c * BC : (c + 1) * BC].reshape(128, E),
            "om": om,
        }
        for c in range(N_CORES)
    ]
    res = run_bass_kernel_spmd(nc, in_maps, core_ids=list(range(N_CORES)), **run_kwargs)
    fg = TILE_F - FD
    full = np.empty((B, NJ, NJ), np.float32)
    for c in range(N_CORES):
        arr = np.asarray(res.results[c]["out"]).reshape(128, NJ * E)
        out3 = np.empty((128, NJ, E), np.float32)
        for it in range(E // TILE_F):
            f0, base = it * TILE_F, it * NJ * TILE_F
            out3[:, :, f0 : f0 + FD] = arr[
                :, base : base + NJ * FD
            ].reshape(128, NJ, FD)
            out3[:, :, f0 + FD : f0 + TILE_F] = arr[
                :, base + NJ * FD : base + NJ * TILE_F
            ].reshape(128, NJ, fg)
        full[c * BC : (c + 1) * BC] = out3.transpose(0, 2, 1).reshape(BC, NJ, NJ)
    if run_kwargs:
        return full, res
    return full



# revision 3
# speedup vs baseline: 1.4693x; 1.4693x over previous
"""Trainium2 Bass kernel for nn_HarmonicOscillatorOrbitals.

out[b, i, j] = exp(-s^2/2) * H_j(s), s = omega * x[b, i, 0], j = 0..31
(physicists' Hermite polynomials), data-parallel over 8 NeuronCores on
the leading batch axis.

Per core, 8192 batches x 32 = 262144 scalars laid out [128, 2048].
The Hermite-times-envelope recurrence

    G_0 = env,  G_1 = t*env,  G_k = t*G_{k-1} - 2(k-1)*G_{k-2}

(t = 2*omega*x) runs entirely on DVE at full width W=2048: per step one
tensor_mul (q = t*G_{k-1}) and one fused scalar_tensor_tensor
(G_k = -c*G_{k-2} + q). GPSIMD is deliberately idle: DVE and GPSIMD
share SBUF ports, and measured concurrent throughput (0.6 elem/ns
combined) is lower than DVE alone (0.76 elem/ns). ACT computes the
envelope; each G_k tile is DMA'd out as soon as written (k-major
[128, 32, 2048] per core), overlapping the serial DVE chain.
"""

from contextlib import ExitStack

import numpy as np

import concourse.bacc as bacc
import concourse.mybir as mybir
import concourse.tile as tile
from concourse.bass_utils import run_bass_kernel_spmd

F32 = mybir.dt.float32
AF = mybir.ActivationFunctionType
ALU = mybir.AluOpType

NJ = 32          # number of Hermite orders
N_CORES = 8
B = 65536        # full batch
BC = B // N_CORES
E = BC * NJ // 128   # 2048 elements per partition per core
W = E                # full-width ops


def _build():
    nc = bacc.Bacc("TRN2", target_bir_lowering=False, debug=False)
    x_d = nc.dram_tensor("x", [128, E], F32, kind="ExternalInput").ap()
    om_d = nc.dram_tensor("om", [1, 1], F32, kind="ExternalInput").ap()
    # k-major output: [128, NJ, E]
    out_d = nc.dram_tensor("out", [128, NJ * E], F32, kind="ExternalOutput").ap()

    with tile.TileContext(nc) as tc, ExitStack() as ctx:
        cpool = ctx.enter_context(tc.tile_pool(name="const", bufs=1))
        gp = ctx.enter_context(tc.tile_pool(name="gp", bufs=1))

        om1 = cpool.tile([128, 1], F32)
        nc.sync.dma_start(om1[0:1, :], om_d[:, :])
        om2 = cpool.tile([128, 1], F32)
        nc.gpsimd.partition_broadcast(om2[:, :], om1[0:1, :])
        nc.scalar.mul(om2[:, :], om2[:, :], 2.0)  # om2 = 2*omega

        x_t = cpool.tile([128, W], F32)
        nc.sync.dma_start(x_t[:, :], x_d[:, :])
        t_t = cpool.tile([128, W], F32)
        nc.scalar.mul(t_t[:, :], x_t[:, :], om2[:, 0:1])  # t = 2*omega*x
        sq = cpool.tile([128, W], F32)
        nc.scalar.activation(sq[:, :], t_t[:, :], AF.Square, scale=0.5)  # s^2

        def g_tile(k):
            return gp.tile([128, W], F32, name=f"g{k}", tag=f"g{k % 6}")

        g = {}
        g[0] = g_tile(0)
        nc.scalar.activation(g[0][:, :], sq[:, :], AF.Exp, scale=-0.5)  # env
        g[1] = g_tile(1)
        nc.vector.tensor_mul(g[1][:, :], t_t[:, :], g[0][:, :])

        def flush(k):
            nc.sync.dma_start(out_d[:, k * E : (k + 1) * E], g[k][:, :])

        flush(0)
        flush(1)

        for k in range(2, NJ):
            c = 2.0 * (k - 1)
            q = gp.tile([128, W], F32, name=f"q{k}", tag=f"q{k % 2}")
            nc.vector.tensor_mul(q[:, :], t_t[:, :], g[k - 1][:, :])
            g[k] = g_tile(k)
            nc.vector.scalar_tensor_tensor(
                g[k][:, :], g[k - 2][:, :], -c, q[:, :], ALU.mult, ALU.add
            )
            flush(k)

    nc.compile()
    return nc


_CACHED_NC = None


def kernel(x: np.ndarray, omega_kernel: np.ndarray, **run_kwargs) -> np.ndarray:
    global _CACHED_NC
    assert x.shape == (B, NJ, 1) and omega_kernel.shape == (1, 1), (
        x.shape,
        omega_kernel.shape,
    )
    x = np.ascontiguousarray(x, np.float32)
    om = np.ascontiguousarray(omega_kernel, np.float32)

    if _CACHED_NC is None:
        _CACHED_NC = _build()
    nc = _CACHED_NC

    in_maps = [
        {
            "x": x[c * BC : (c + 1) * BC].reshape(128, E),
            "om": om,
        }
        for c in range(N_CORES)
    ]
    res = run_bass_kernel_spmd(nc, in_maps, core_ids=list(range(N_CORES)), **run_kwargs)
    full = np.empty((B, NJ, NJ), np.float32)
    for c in range(N_CORES):
        arr = np.asarray(res.results[c]["out"]).reshape(128, NJ, 64, NJ)
        # [p, k, b2, i] -> [p, b2, i, k]
        full[c * BC : (c + 1) * BC] = arr.transpose(0, 2, 3, 1).reshape(BC, NJ, NJ)
    if run_kwargs:
        return full, res
    return full
